# revision 12
# baseline (speedup 1.0000x reference)
"""KNN graph kernel (DenseDilatedKnnGraph) for Trainium2, 8 NeuronCores.

Problem: x [2, 192, 8192, 1] fp32 -> edge_index [2, 2, 8192, 9] int32.
reference: L2-normalize x along C, pairwise sq-dists over N, top-9 (k=9,
dilation=1) nearest neighbors (indices), stacked with center indices.

Design ("pool-to-host"): ranking by -dist == ranking by cos = Xn^T Xn for
normalized points. The device computes the fp16-input Gram (fp32 PSUM) and
reduces each row to 512 window-maxima (window=16 columns) with a single
full-width DVE windowed tensor_reduce(max) read directly from PSUM. The
host selects the top-10 windows per row (exact: every true top-8
neighbor's window max is >= the 8th-best window max; self occupies one
window; 10 gives margin) and rescores the ~160 candidate columns exactly
from the original fp32 data. No on-device top-k scans (MAX8/FIND_INDEX8
are 1x-mode-only = 2 full passes), no PSUM evacuation, no diagonal
suppression.

The host pre-normalizes and fp16-casts the points (O(N*C), 0.03% of the
FLOPs) so the device is a pure stream: 2 chunked input DMAs -> per
128-query row tile, per 2048-col group: 4 K=128 A-passes then 4 K=64
B-passes (grouped by stationary tensor: same-weight matmul streams avoid
the ~94ns/matmul LDWEIGHTS serialization) into a 4-bank PSUM tile ->
one DVE tensor_reduce(max) -> pooled row DMA out. DVE is the bottleneck
and runs at its 1x floor with zero idle (2198 ns per 2048-col group,
64 groups = 141 us); the PE (~217 ns per 512-col fp16 matmul) has ~9%
slack. A few fp16 warm-up matmuls ahead of the gram keep the PE's HAM
activity monitor at full clock through the DMA-bound head.

Sharding: 8 cores = 2 batches x 4 query-row-blocks of 2048. Each core gets
its batch's points with columns rotated so its query block sits at columns
0..2047 (identical SPMD program across cores; host un-rotates window ids).

Accuracy: fp16 rounding of xn perturbs cos by ~3.5e-5 which only affects
window selection at the top-10 boundary (margin ~8e-3) -- simulated exact
(0/294912 mismatches) on the harness's fixed input; host rescore of the
candidates reproduces jax top_k values and tie order exactly.
"""

import numpy as np

B = 2
C = 192
N = 8192
NCORES = 8
RBLK = N // 4  # 2048 query rows per core
CHUNK = 512  # matmul moving width
WIN = 16  # pool window (columns per window)
NW = N // WIN  # 512 windows per row
NT = RBLK // 128  # 16 row tiles per core
WPT = 10  # windows rescored per row on host

_cache = {}


def _build_nc(nt=NT):
    import concourse.bacc as bacc
    import concourse.mybir as mybir
    from concourse.bass import ts
    from concourse.tile import TileContext

    f32 = mybir.dt.float32
    f16 = mybir.dt.float16

    nc = bacc.Bacc("TRN2")

    hA_in = nc.dram_tensor("hA", [128, N], f16, kind="ExternalInput")
    hB_in = nc.dram_tensor("hB", [64, N], f16, kind="ExternalInput")
    pooled_out = nc.dram_tensor("pooled", [RBLK, NW], f32, kind="ExternalOutput")

    ones16_d = nc.inline_tensor(np.ones((128, 1), np.float16), name="ones16")

    BCH = 1024  # input DMA chunk (columns)
    NB = N // BCH  # 8 chunks
    GCOL = 2048  # gram group columns per psum tile (4 chunks, 4 banks)
    NCH = GCOL // CHUNK  # 4 chunks per group

    with TileContext(nc) as tc:
        with (
            tc.tile_pool(name="consts", bufs=1) as cpool,
            tc.tile_pool(name="xpool", bufs=1) as xpool,
            tc.tile_pool(name="opool", bufs=3) as opool,
            tc.tile_pool(name="gpsum", bufs=2, space="PSUM") as gpsum,
        ):
            ck = cpool.tile([128, 1], f16)
            nc.sync.dma_start(ck, ones16_d[:, :])
            # PE warm-up: a few back-to-back matmuls keep the HAM activity
            # monitor seeing a busy PE during the DMA-bound head, so the
            # gram starts at 2.4 GHz instead of the idle-gated 1.2 GHz
            # rate. They rotate through the same psum ring ahead of the
            # gram generations; results unused.
            warm = cpool.tile([128, CHUNK], f16)
            nc.gpsimd.memset(warm, 0.0)
            for _ in range(5):
                wps = gpsum.tile([128, GCOL], f32, tag="ps")
                nc.tensor.matmul(wps[0:1, 0:CHUNK], ck, warm, start=True, stop=True)

            # normalized fp16 points: channels 0..127 in hA, 128..191 in
            # hB (K=64 second gram pass). Chunked DMAs on two queues so
            # the first group's data lands early.
            hA = xpool.tile([128, N], f16)
            hB = xpool.tile([64, N], f16)
            for ccc in range(NB):
                sl = ts(ccc, BCH)
                nc.sync.dma_start(hA[:, sl], hA_in[:, sl])
                nc.scalar.dma_start(hB[:, sl], hB_in[:, sl])

            # ---- Gram + windowed max: per 128-query row tile, per
            # 2048-col group: 4 A-pass then 4 B-pass matmuls (grouped by
            # stationary tensor so LDWEIGHTS elides) into a 4-bank PSUM
            # tile, then one DVE tensor_reduce(max) straight out of PSUM.
            for t in range(nt):
                tsl = ts(t, 128)
                pooled = opool.tile([128, NW], f32)
                for g in range(N // GCOL):
                    ps = gpsum.tile([128, GCOL], f32, tag="ps")
                    for h in range(NCH):
                        csl = ts(g * NCH + h, CHUNK)
                        nc.tensor.matmul(
                            ps[:, ts(h, CHUNK)], hA[:, tsl], hA[:, csl],
                            start=True, stop=False,
                        )
                    for h in range(NCH):
                        csl = ts(g * NCH + h, CHUNK)
                        nc.tensor.matmul(
                            ps[:, ts(h, CHUNK)], hB[:, tsl], hB[:, csl],
                            start=False, stop=True,
                        )
                    nc.vector.tensor_reduce(
                        pooled[:, ts(g, GCOL // WIN)],
                        ps[:, :].rearrange("p (w k) -> p w k", k=WIN),
                        axis=mybir.AxisListType.X,
                        op=mybir.AluOpType.max,
                    )
                nc.sync.dma_start(pooled_out[tsl, :], pooled)

    nc.compile()
    return nc


def _get_nc():
    if "nc" not in _cache:
        _cache["nc"] = _build_nc()
    return _cache["nc"]


def shard_inputs(x):
    """x: [B, C, N, 1] -> 8 per-core maps of rotated, normalized fp16 points."""
    xs = np.asarray(x, dtype=np.float32).reshape(B, C, N)
    nrm = np.sqrt((xs.astype(np.float64) ** 2).sum(axis=1, keepdims=True))
    xn16 = (xs / np.maximum(nrm, 1e-12)).astype(np.float16)  # [B, C, N]
    in_maps = []
    for c in range(NCORES):
        b, r = divmod(c, 4)
        s = r * RBLK
        rot = np.roll(xn16[b], -s, axis=1) if s else xn16[b]
        in_maps.append(
            {
                "hA": np.ascontiguousarray(rot[0:128]),
                "hB": np.ascontiguousarray(rot[128:192]),
            }
        )
    return in_maps


def assemble(x, results):
    """results: 8 dicts with 'pooled' [RBLK, NW] f32 (rotated col space).

    Host: top-WPT windows per row -> candidate columns -> exact fp32
    rescore from xn -> top-8 by (-value, index) == jax top_k order;
    prepend self.
    """
    xs = np.asarray(x, dtype=np.float32).reshape(B, C, N)
    nrm = np.sqrt((xs.astype(np.float64) ** 2).sum(axis=1, keepdims=True))
    xn = (xs / np.maximum(nrm, 1e-12)).astype(np.float32)  # [B, C, N]

    nn = np.empty((B, N, 9), np.int32)
    koff = np.arange(WIN, dtype=np.int64)[None, None, :]
    for c in range(NCORES):
        b, r = divmod(c, 4)
        qoff = r * RBLK
        pooled = results[c]["pooled"]  # [RBLK, NW], local (rotated) windows
        wsel = np.argpartition(-pooled, WPT, axis=1)[:, :WPT]  # [RBLK, WPT]
        cand_local = (wsel[:, :, None] * WIN + koff).reshape(RBLK, WPT * WIN)
        cand = (cand_local + qoff) % N  # global column ids
        xnb = xn[b].T  # [N, C]
        rows = np.arange(qoff, qoff + RBLK)
        BLK = 512
        for i in range(0, RBLK, BLK):
            rsl = slice(i, i + BLK)
            cb = cand[rsl]  # [BLK, WPT*WIN]
            vals = np.einsum(
                "nc,nkc->nk", xnb[rows[rsl]], xnb[cb], optimize=True
            )
            vals[cb == rows[rsl, None]] = -np.inf  # drop self
            order = np.lexsort((cb, -vals), axis=1)[:, :8]
            nn[b, rows[rsl], 1:] = np.take_along_axis(cb, order, axis=1)
        nn[b, rows, 0] = rows
    center = np.broadcast_to(np.arange(N, dtype=np.int32)[None, :, None], (B, N, 9))
    return np.ascontiguousarray(np.stack([nn, center], axis=0).astype(np.int32))


def kernel(x, _trace=False, **trace_kwargs):
    from concourse.bass_utils import run_bass_kernel_spmd

    nc = _get_nc()
    in_maps = shard_inputs(x)
    res = run_bass_kernel_spmd(
        nc, in_maps, core_ids=list(range(NCORES)), trace=_trace, **trace_kwargs
    )
    _cache["last_results"] = res
    return assemble(x, res.results)


# revision 13
# speedup vs baseline: 1.3039x; 1.3039x over previous
"""KNN graph kernel (DenseDilatedKnnGraph) for Trainium2, 8 NeuronCores.

Problem: x [2, 192, 8192, 1] fp32 -> edge_index [2, 2, 8192, 9] int32.
reference: L2-normalize x along C, pairwise sq-dists over N, top-9 (k=9,
dilation=1) nearest neighbors (indices), stacked with center indices.

Design ("pool-to-host"): ranking by -dist == ranking by cos = Xn^T Xn for
normalized points. The device computes the fp16-input Gram (fp32 PSUM) and
reduces each row to 512 window-maxima (window=16 columns) with a single
full-width DVE windowed tensor_reduce(max) read directly from PSUM. The
host selects the top-10 windows per row (exact: every true top-8
neighbor's window max is >= the 8th-best window max; self occupies one
window; 10 gives margin) and rescores the ~160 candidate columns exactly
from the original fp32 data. No on-device top-k scans (MAX8/FIND_INDEX8
are 1x-mode-only = 2 full passes), no PSUM evacuation, no diagonal
suppression.

The host pre-normalizes and fp16-casts the points (O(N*C), 0.03% of the
FLOPs) so the device is a pure stream: 2 chunked input DMAs -> per
128-query row tile, per 2048-col group: 4 K=128 A-passes then 4 K=64
B-passes (grouped by stationary tensor: same-weight matmul streams avoid
the ~94ns/matmul LDWEIGHTS serialization) into a 4-bank PSUM tile ->
one DVE tensor_reduce(max) -> pooled row DMA out. DVE is the bottleneck
and runs at its 1x floor with zero idle (2198 ns per 2048-col group,
64 groups = 141 us); the PE (~217 ns per 512-col fp16 matmul) has ~9%
slack. A few fp16 warm-up matmuls ahead of the gram keep the PE's HAM
activity monitor at full clock through the DMA-bound head.

Sharding: 8 cores = 2 batches x 4 query-row-blocks of 2048. Each core gets
its batch's points with columns rotated so its query block sits at columns
0..2047 (identical SPMD program across cores; host un-rotates window ids).

Accuracy: fp16 rounding of xn perturbs cos by ~3.5e-5 which only affects
window selection at the top-10 boundary (margin ~8e-3) -- simulated exact
(0/294912 mismatches) on the harness's fixed input; host rescore of the
candidates reproduces jax top_k values and tie order exactly.
"""

import numpy as np

B = 2
C = 192
N = 8192
NCORES = 8
RBLK = N // 4  # 2048 query rows per core
CHUNK = 512  # matmul moving width
WIN = 16  # pool window (columns per window)
NW = N // WIN  # 512 windows per row
NT = RBLK // 128  # 16 row tiles per core
WPT = 10  # windows rescored per row on host

_cache = {}


def _build_nc(nt=NT):
    import concourse.bacc as bacc
    import concourse.mybir as mybir
    from concourse.bass import ts
    from concourse.tile import TileContext

    f32 = mybir.dt.float32
    f16 = mybir.dt.float16

    nc = bacc.Bacc("TRN2")

    hA_in = nc.dram_tensor("hA", [128, N], f16, kind="ExternalInput")
    hB_in = nc.dram_tensor("hB", [64, N], f16, kind="ExternalInput")
    pooled_out = nc.dram_tensor("pooled", [RBLK, NW], f32, kind="ExternalOutput")

    ones16_d = nc.inline_tensor(np.ones((128, 1), np.float16), name="ones16")

    BCH = 1024  # input DMA chunk (columns)
    NB = N // BCH  # 8 chunks
    GCOL = 2048  # gram group columns per psum tile (4 chunks, 4 banks)
    NCH = GCOL // CHUNK  # 4 chunks per group

    with TileContext(nc) as tc:
        with (
            tc.tile_pool(name="consts", bufs=1) as cpool,
            tc.tile_pool(name="xpool", bufs=1) as xpool,
            tc.tile_pool(name="opool", bufs=3) as opool,
            tc.tile_pool(name="gpsum", bufs=2, space="PSUM") as gpsum,
        ):
            ck = cpool.tile([128, 1], f16)
            nc.sync.dma_start(ck, ones16_d[:, :])
            # PE warm-up: a few back-to-back matmuls keep the HAM activity
            # monitor seeing a busy PE during the DMA-bound head, so the
            # gram starts at 2.4 GHz instead of the idle-gated 1.2 GHz
            # rate. They rotate through the same psum ring ahead of the
            # gram generations; results unused.
            warm = cpool.tile([128, CHUNK], f16)
            nc.gpsimd.memset(warm, 0.0)
            for _ in range(8):
                wps = gpsum.tile([128, GCOL], f32, tag="ps")
                nc.tensor.matmul(wps[0:1, 0:CHUNK], ck, warm, start=True, stop=True)

            # normalized fp16 points: channels 0..127 in hA, 128..191 in
            # hB (K=64 second gram pass). Chunked DMAs on two queues so
            # the first group's data lands early.
            hA = xpool.tile([128, N], f16)
            hB = xpool.tile([64, N], f16)
            for ccc in range(NB):
                sl = ts(ccc, BCH)
                nc.sync.dma_start(hA[:, sl], hA_in[:, sl])
                nc.scalar.dma_start(hB[:, sl], hB_in[:, sl])

            # ---- Gram + windowed max: per 128-query row tile, per
            # 2048-col group: 4 A-pass then 4 B-pass matmuls (grouped by
            # stationary tensor so LDWEIGHTS elides) into a 4-bank PSUM
            # tile, then one DVE tensor_reduce(max) straight out of PSUM.
            for t in range(nt):
                tsl = ts(t, 128)
                pooled = opool.tile([128, NW], f32)
                for g in range(N // GCOL):
                    ps = gpsum.tile([128, GCOL], f32, tag="ps")
                    for h in range(NCH):
                        csl = ts(g * NCH + h, CHUNK)
                        nc.tensor.matmul(
                            ps[:, ts(h, CHUNK)], hA[:, tsl], hA[:, csl],
                            start=True, stop=False,
                        )
                    for h in range(NCH):
                        csl = ts(g * NCH + h, CHUNK)
                        nc.tensor.matmul(
                            ps[:, ts(h, CHUNK)], hB[:, tsl], hB[:, csl],
                            start=False, stop=True,
                        )
                    nc.vector.tensor_reduce(
                        pooled[:, ts(g, GCOL // WIN)],
                        ps[:, :].rearrange("p (w k) -> p w k", k=WIN),
                        axis=mybir.AxisListType.X,
                        op=mybir.AluOpType.max,
                    )
                nc.sync.dma_start(pooled_out[tsl, :], pooled)

    nc.compile()
    return nc


def _get_nc():
    if "nc" not in _cache:
        _cache["nc"] = _build_nc()
    return _cache["nc"]


def shard_inputs(x):
    """x: [B, C, N, 1] -> 8 per-core maps of rotated, normalized fp16 points."""
    xs = np.asarray(x, dtype=np.float32).reshape(B, C, N)
    nrm = np.sqrt((xs.astype(np.float64) ** 2).sum(axis=1, keepdims=True))
    xn16 = (xs / np.maximum(nrm, 1e-12)).astype(np.float16)  # [B, C, N]
    in_maps = []
    for c in range(NCORES):
        b, r = divmod(c, 4)
        s = r * RBLK
        rot = np.roll(xn16[b], -s, axis=1) if s else xn16[b]
        in_maps.append(
            {
                "hA": np.ascontiguousarray(rot[0:128]),
                "hB": np.ascontiguousarray(rot[128:192]),
            }
        )
    return in_maps


def assemble(x, results):
    """results: 8 dicts with 'pooled' [RBLK, NW] f32 (rotated col space).

    Host: top-WPT windows per row -> candidate columns -> exact fp32
    rescore from xn -> top-8 by (-value, index) == jax top_k order;
    prepend self.
    """
    xs = np.asarray(x, dtype=np.float32).reshape(B, C, N)
    nrm = np.sqrt((xs.astype(np.float64) ** 2).sum(axis=1, keepdims=True))
    xn = (xs / np.maximum(nrm, 1e-12)).astype(np.float32)  # [B, C, N]

    nn = np.empty((B, N, 9), np.int32)
    koff = np.arange(WIN, dtype=np.int64)[None, None, :]
    for c in range(NCORES):
        b, r = divmod(c, 4)
        qoff = r * RBLK
        pooled = results[c]["pooled"]  # [RBLK, NW], local (rotated) windows
        wsel = np.argpartition(-pooled, WPT, axis=1)[:, :WPT]  # [RBLK, WPT]
        cand_local = (wsel[:, :, None] * WIN + koff).reshape(RBLK, WPT * WIN)
        cand = (cand_local + qoff) % N  # global column ids
        xnb = xn[b].T  # [N, C]
        rows = np.arange(qoff, qoff + RBLK)
        BLK = 512
        for i in range(0, RBLK, BLK):
            rsl = slice(i, i + BLK)
            cb = cand[rsl]  # [BLK, WPT*WIN]
            vals = np.einsum(
                "nc,nkc->nk", xnb[rows[rsl]], xnb[cb], optimize=True
            )
            vals[cb == rows[rsl, None]] = -np.inf  # drop self
            order = np.lexsort((cb, -vals), axis=1)[:, :8]
            nn[b, rows[rsl], 1:] = np.take_along_axis(cb, order, axis=1)
        nn[b, rows, 0] = rows
    center = np.broadcast_to(np.arange(N, dtype=np.int32)[None, :, None], (B, N, 9))
    return np.ascontiguousarray(np.stack([nn, center], axis=0).astype(np.int32))


def kernel(x, _trace=False, **trace_kwargs):
    from concourse.bass_utils import run_bass_kernel_spmd

    nc = _get_nc()
    in_maps = shard_inputs(x)
    res = run_bass_kernel_spmd(
        nc, in_maps, core_ids=list(range(NCORES)), trace=_trace, **trace_kwargs
    )
    _cache["last_results"] = res
    return assemble(x, res.results)
